# revision 23
# baseline (speedup 1.0000x reference)
"""Squared-L2 distance retrieval kernel (logits[q,p] = ||proto[p]-query[q]||^2)
for Trainium2, data-parallel over 8 NeuronCores, written in RAW BASS (no
TileContext) with fully manual semaphore wiring.

Math per core (256-query shard, proto replicated), identical to the validated
Tile baseline: logits = -2*(qp - q2/2 - p2/2) as one PSUM chain per 128-query
tile (fp8 matmuls, host-prepacked transposed operands), ||q||^2 via squares on
ACT/DVE/Pool reduced by narrow [128,4] PE matmuls, -p2/2 prepacked as two fp8
bias columns (hi/lo of -p2/8) closed by a K=2 matmul against constant 4.0,
copyback = one DVE tensor_scalar per tile.

Why raw bass: the Tile framework forces the output through dma_start (HWDGE
desc-gen 625ns + DGE delay 650ns after the last copyback) and wraps the kernel
in entry/exit drain cascades. Manual sems enable kv_writeback(prepare_only) +
trigger_dma for the output: descriptors are generated on the Pool engine ~2us
before the data exists, and the final trigger goes straight to the wire. Tile
cannot express this (its DMASW-lane accounting requires the prep's completion
sem to be the lane sem, which walrus rejects on prepare-only descriptors;
manual sems + an explicit final wait_ge sidestep both that and the Tile exit
drain deadlock).

Validated on hw at 5887 ns (TimelineSim == graded HW exec time), rel err
7.8e-3, vs 8072 ns for the Tile version of the same math. Budget:
  ~616   bass preamble (4 const-pool memsets on Pool + all-engine barrier;
         emitted by Bass.__init__, untouchable)
  ~3180  query tile0 sem: 25 seq + 625 HWDGE gen + 650 DGE delay + 364 wire
         + 900 SEM_PROP_DMA. Tile1 lands via Pool SWDGE (desc-gen in
         parallel with HWDGE), proto+bias last on SP; their sems (3576/3781)
         stay off the critical path.
  ~1130  squares on ACT/DVE/Pool, split tuned by TimelineSim sweeps (~500
         configs, 8-col granularity) so the three per-tile-1 square sems
         land within ~15ns of each other (~4305..4320)
  ~1572  tail: q2c-t1 narrow matmuls (+173 PE SBUF ack) -> DVE copyback 192
         (+125 PSUM ack) -> trigger (no desc-gen, no DGE delay) -> 13 wire
         -> 900 SEM_PROP_DMA -> SP final wait.
Paths examined and rejected (with the blocking reason):
  - dma_gather prep+trigger for the input: the executor demands a full
    [128,8] int16 index tensor with all values in range, and the iota(+sem)
    setup on Pool delays the SWDGE desc-gen by more than the 650ns DGE delay
    it avoids; also carries the hw row-rotation quirk ((j-16)%128).
  - splitting the first DMA finer: every extra HWDGE DMA costs 625 serial
    desc-gen + 650 DGE; any 4th input DMA pushes proto's sem past 4200.
  - folding q2 into the qp PSUM chain via ones-rhs matmuls: +200ns of 27ns
    64-col matmuls on the PE critical path vs 2ns 4-col q2c matmuls.
  - copyback on ACT / split copybacks: ACT cannot read the q2c scalar from
    PSUM (activation bias must be SBUF) and its 370ns access init loses to
    DVE even when DVE is busy; two half copybacks serialize on DVE anyway.
  - remote_dma sem-only descriptor after the writeback in the same queue to
    dodge the output's 900ns sem prop (model shows ~650ns win): no hardware
    ordering guarantee between writeback and RDMA descriptors — a host-read
    race; rejected for correctness.
"""

import numpy as np

B, P, D = 1, 64, 1024
Q = 2048
N_CORES = 8
QSH = Q // N_CORES   # 256 query rows per core
NT = QSH // 128      # m-tiles per core
ND = D // 128        # contraction chunks

PTO = ND * P              # proto block width (512)
P2O = PTO + P             # end of p2 block / start of query block (576)
QW = NT * ND * 128        # query block width (2048)
XW = P2O + QW             # total input width (2624)

_cache = {}

CFG = dict(
    dtype="f8e4",          # matmul operand dtype
    n_warmup=4,            # dummy PE matmuls to climb the clock ramp
    out_path="trigger",    # "trigger" = kv_writeback prep + trigger_dma
                           # "dma"     = plain SP HWDGE dma_start fallback
    # per-tile square split: tile -> ((engine, col_lo, col_hi), ...) in cols
    sq_split=(
        (("act", 0, 448), ("dve", 448, 828), ("pool", 828, 1024)),
        (("act", 0, 236), ("dve", 236, 764), ("pool", 764, 1024)),
    ),
)

SAFE_CFG = dict(CFG, out_path="dma")


def _mm_dt(cfg):
    import concourse.mybir as mybir

    return {"bf16": mybir.dt.bfloat16, "f8e4": mybir.dt.float8e4}[cfg["dtype"]]


def _build_nc(cfg=None):
    import concourse.mybir as mybir
    from concourse import bacc

    cfg = dict(CFG, **(cfg or {}))
    f32 = mybir.dt.float32
    bf16 = mybir.dt.bfloat16
    i32 = mybir.dt.int32
    mdt = _mm_dt(cfg)
    Alu = mybir.AluOpType

    nc = bacc.Bacc("TRN2", target_bir_lowering=False, debug=False)
    sp, ve, sc, gp, pe = nc.sync, nc.vector, nc.scalar, nc.gpsimd, nc.tensor

    # Input: query block and proto+bias block as separate DRAM tensors.
    q_in = nc.dram_tensor("qT8", [128, QW], mdt, kind="ExternalInput").ap()
    p_in = nc.dram_tensor("pT8", [128, P2O], mdt, kind="ExternalInput").ap()
    if cfg["out_path"] == "trigger":
        # kv_writeback layout [batch, d_head_inner, d_head_outer, n_ctx]
        logits = nc.dram_tensor("logitsP", [1, 128, 1, NT * P], f32,
                                kind="ExternalOutput")
    else:
        logits = nc.dram_tensor("logitsP", [128, NT, P], f32,
                                kind="ExternalOutput")

    # --- SBUF ---
    xt = nc.alloc_sbuf_tensor("xt", [128, XW], mdt)
    qsq = nc.alloc_sbuf_tensor("qsq", [128, QW], bf16)
    out_sb = nc.alloc_sbuf_tensor("out_sb", [128, NT * P], f32)
    ones4 = nc.alloc_sbuf_tensor("ones4", [128, 4], bf16)
    fours = nc.alloc_sbuf_tensor("fours", [2, 128], mdt)
    kvi = nc.alloc_sbuf_tensor("kvi", [128, 1], i32)
    wrm = nc.alloc_sbuf_tensor("wrm", [128, 64], bf16)

    # --- PSUM ---
    wps = nc.alloc_psum_tensor("wps", [64, 64], f32)
    acc = [nc.alloc_psum_tensor(f"acc{t}", [128, P], f32) for t in range(NT)]
    q2c = [nc.alloc_psum_tensor(f"q2c{t}", [128, 4], f32) for t in range(NT)]

    # --- semaphores ---
    s_q = [nc.alloc_semaphore(f"s_q{t}") for t in range(NT)]   # query tile DMAs
    s_pr = nc.alloc_semaphore("s_pr")                          # proto+bias DMA
    s_const = nc.alloc_semaphore("s_const")                    # DVE memsets
    s_sq = [nc.alloc_semaphore(f"s_sq{t}") for t in range(NT)]  # squares (+1 each)
    s_q2c = [nc.alloc_semaphore(f"s_q2c{t}") for t in range(NT)]
    s_cb = nc.alloc_semaphore("s_cb")                          # copybacks
    s_prep = nc.alloc_semaphore("s_prep")                      # output desc-gen
    s_out = nc.alloc_semaphore("s_out")                        # output DMA done

    def pts(d):
        return xt[:, d * P:(d + 1) * P]

    def qcols(t, clo, chi):
        return xt[:, P2O + t * ND * 128 + clo:P2O + t * ND * 128 + chi]

    def qsqcols(t, clo, chi):
        return qsq[:, t * ND * 128 + clo:t * ND * 128 + chi]

    # --- input: query tile0 first on the wire (it gates the squares, SP
    # HWDGE), query tile1 on the Pool SWDGE lane (desc-gen in parallel),
    # proto+bias last on SP (its consumer, the PE matmul stream, has slack) ---
    sp.dma_start(qcols(0, 0, ND * 128),
                 q_in[:, 0:ND * 128]).then_inc(s_q[0], 16)
    gp.dma_start(qcols(1, 0, ND * 128),
                 q_in[:, ND * 128:]).then_inc(s_q[1], 16)
    sp.dma_start(xt[:, :P2O], p_in).then_inc(s_pr, 16)

    # --- constants on DVE (done during the DMA latency window) ---
    ve.memset(kvi[:], 0)
    ve.memset(ones4[:], 1.0)
    ve.memset(fours[:], 4.0)
    ve.memset(wrm[:], -0.5).then_inc(s_const, 1)

    # --- output descriptor pre-generation on Pool (after the input desc-gen;
    # both are long done before the copybacks) ---
    if cfg["out_path"] == "trigger":
        gp.wait_ge(s_const, 1)
        gp.kv_writeback(
            logits[:, :, :, :],
            out_sb[:].rearrange("p (a b c) -> p a b c", a=1, b=1),
            kvi[:], prepare_only=True, sem=s_out, queue_num=0,
        ).then_inc(s_prep, 1)

    # --- PE warmup during the DMA latency window ---
    pe.wait_ge(s_const, 1)
    for _ in range(cfg["n_warmup"]):
        pe.matmul(wps[:], wrm[:, :64], wrm[:, :64], start=True, stop=True)

    # --- squares, as each query tile lands ---
    def emit_square(e, dst, src):
        if e == "act":
            sc.wait_ge(s_q[t], 16)
            return sc.square(dst, src)
        if e == "dve":
            ve.wait_ge(s_q[t], 16)
            return ve.tensor_tensor(out=dst, in0=src, in1=src, op=Alu.mult)
        gp.wait_ge(s_q[t], 16)
        return gp.tensor_tensor(out=dst, in0=src, in1=src, op=Alu.mult)

    for t in range(NT):
        for e, clo, chi in cfg["sq_split"][t]:
            emit_square(e, qsqcols(t, clo, chi), qcols(t, clo, chi)).then_inc(
                s_sq[t], 1)

    # --- PE chains: acc-t0, q2c-t0, acc-t1, q2c-t1 (each chain's readiness
    # is nondecreasing in this order, so the in-order SEQ never head-blocks) ---
    def acc_chain(t):
        if t == 0:
            pe.wait_ge(s_pr, 16)
        pe.wait_ge(s_q[t], 16)
        for d in range(ND):
            pe.matmul(acc[t][:], qcols(t, d * 128, (d + 1) * 128), pts(d),
                      start=(d == 0), stop=False)
        # -p2/2 broadcast closes the chain: 4 x (-p2/8 hi/lo)
        pe.matmul(acc[t][:], fours[:], xt[0:2, PTO:PTO + P],
                  start=False, stop=True)

    def q2c_chain(t):
        pe.wait_ge(s_sq[t], len(cfg["sq_split"][t]))
        for d in range(ND):
            mm = pe.matmul(q2c[t][:], qsqcols(t, d * 128, (d + 1) * 128),
                           ones4[:], start=(d == 0), stop=(d == ND - 1))
        mm.then_inc(s_q2c[t], 1)

    acc_chain(0)
    q2c_chain(0)
    acc_chain(1)
    q2c_chain(1)

    # --- copybacks on DVE: out = -2*(qp - p2/2) + q2 (q2 read from PSUM).
    # PE retires in order, so s_q2c[t] also implies acc[t] is complete. ---
    for t in range(NT):
        ve.wait_ge(s_q2c[t], 1)
        ve.tensor_scalar(out_sb[:, t * P:(t + 1) * P], acc[t][:], -2.0,
                         q2c[t][:, 0:1], op0=Alu.mult,
                         op1=Alu.add).then_inc(s_cb, 1)

    # --- output: fire the prepared descriptors; no desc-gen, no DGE delay ---
    if cfg["out_path"] == "trigger":
        gp.wait_ge(s_prep, 1)
        gp.trigger_dma(count=1, queue_num=0)._wait_ge(s_cb, NT)
    else:
        sp.wait_ge(s_cb, NT)
        sp.dma_start(
            logits[:, :, :],
            out_sb[:].rearrange("p (t q) -> p t q", t=NT)).then_inc(s_out, 16)

    # the kernel is complete only once the output DMA's completion sem fires
    sp.wait_ge(s_out, 16)

    nc.compile()
    return nc


def _core_inputs(query, proto, cfg=None):
    cfg = dict(CFG, **(cfg or {}))
    npdt = {"bf16": "bfloat16", "f8e4": "float8_e4m3"}[cfg["dtype"]]
    import ml_dtypes

    npdt = np.dtype(getattr(ml_dtypes, npdt))
    # proto block + -p2/8 hi/lo bias block (shared across cores)
    head = np.zeros((128, P2O), dtype=npdt)
    head[:, :PTO] = proto.reshape(P, ND, 128).transpose(2, 1, 0).reshape(
        128, PTO).astype(npdt)
    p2q = -0.125 * (proto.astype(np.float64) ** 2).sum(-1)
    hi = p2q.astype(npdt)
    head[0, PTO:PTO + P] = hi
    head[1, PTO:PTO + P] = (p2q - hi.astype(np.float64)).astype(npdt)
    maps = []
    for c in range(N_CORES):
        shard = query[c * QSH:(c + 1) * QSH]
        # qT8[dp, (t*ND + d)*128 + q] = shard[t*128 + q, d*128 + dp]
        qk = shard.reshape(NT, 128, ND, 128).transpose(
            3, 0, 2, 1).reshape(128, QW).astype(npdt)
        maps.append({"qT8": np.ascontiguousarray(qk),
                     "pT8": np.ascontiguousarray(head)})
    return maps


def _unpack_out(res):
    # logitsP[.., p, .., t*64+c] = logits[t*128+p, c]
    r = np.asarray(res).reshape(128, NT, P)
    return np.ascontiguousarray(r.transpose(1, 0, 2).reshape(QSH, P))


def _get_nc():
    if "nc" not in _cache:
        _cache["nc"] = _build_nc()
    return _cache["nc"]


def kernel(**inputs) -> np.ndarray:
    from concourse.bass_utils import run_bass_kernel_spmd

    query = np.ascontiguousarray(
        np.asarray(inputs["query"], dtype=np.float32).reshape(Q, D))
    proto = np.asarray(inputs["proto"], dtype=np.float32).reshape(P, D)

    nc = _get_nc()
    in_maps = _core_inputs(query, proto)
    res = run_bass_kernel_spmd(nc, in_maps, core_ids=list(range(N_CORES)))
    return np.concatenate(
        [_unpack_out(r["logitsP"]) for r in res.results], axis=0)
